# revision 9
# baseline (speedup 1.0000x reference)
import sys

sys.path.insert(0, "/opt/trn_rl_repo")
import numpy as np
import ml_dtypes

import concourse.bass as bass
import concourse.bacc as bacc
import concourse.mybir as mybir
import concourse.tile as tile
from concourse.bass import IndirectOffsetOnAxis
from concourse.bass_utils import run_bass_kernel_spmd
from concourse.masks import make_identity

BF16 = mybir.dt.bfloat16
F32 = mybir.dt.float32

N_CORES = 8
B, N, E, D, T, L = 2, 50000, 250000, 64, 16, 3
NPC = N // N_CORES          # nodes per core = 6250
TILES = (NPC + 127) // 128  # node tiles per core = 49
NPAD = TILES * 128          # padded nodes per core = 6272


def _host_prep(x, edge_index, edge_attr, pv_W, pv_b, pe_W, pe_b):
    """Input projections + edge bucketing/padding. Returns per-core arrays."""
    x = np.asarray(x, np.float32)
    src = np.asarray(edge_index[0], np.int64)
    dst = np.asarray(edge_index[1], np.int64)
    ea = np.asarray(edge_attr, np.float32)

    h0 = np.maximum(x @ np.asarray(pv_W, np.float32) + np.asarray(pv_b, np.float32), 0.0)
    g0 = np.maximum(ea @ np.asarray(pe_W, np.float32) + np.asarray(pe_b, np.float32), 0.0)

    # remapped global-table row for node n: core*NPAD + local
    src_row = ((src // NPC) * NPAD + (src % NPC)).astype(np.int32)

    core = dst // NPC
    local = dst - core * NPC
    tl = local // 128
    rel = local - tl * 128

    counts = np.zeros((N_CORES, TILES), np.int64)
    for r in range(N_CORES):
        counts[r] = np.bincount(tl[core == r], minlength=TILES)
    K = max(1, int(np.max((counts + 127) // 128)))  # chunks per tile (uniform)
    C = TILES * K  # chunks per core

    idx_all, oh_all, g0_all = [], [], []
    for r in range(N_CORES):
        m = np.nonzero(core == r)[0]
        em = m[np.argsort(tl[m], kind="stable")]
        tlr = tl[em]
        slots = C * 128
        s_src = np.zeros(slots, np.int32)
        s_rel = np.full(slots, 255, np.int32)
        s_g0 = np.zeros((slots, D), np.float32)
        for t in range(TILES):
            sel = em[tlr == t]
            base = t * K * 128
            s_src[base : base + len(sel)] = src_row[sel]
            s_rel[base : base + len(sel)] = rel[sel]
            s_g0[base : base + len(sel)] = g0[sel]
        oh = np.zeros((C, 128, 128), np.float32)
        cc = np.arange(slots) // 128
        ee = np.arange(slots) % 128
        real = s_rel < 255
        oh[cc[real], ee[real], s_rel[real]] = 1.0
        idx_all.append(s_src.reshape(C, 128).T.copy())  # [128, C] int32
        oh_all.append(oh.transpose(1, 0, 2).reshape(128, C * 128).astype(ml_dtypes.bfloat16))
        gch = s_g0.reshape(C, 128, D).transpose(2, 0, 1).reshape(D, C * 128)
        g0_all.append(np.concatenate([gch, gch], axis=0).astype(ml_dtypes.bfloat16))

    # h table [8*NPAD, 2*D] f32
    table = np.zeros((N_CORES * NPAD, 2 * D), np.float32)
    for r in range(N_CORES):
        for b in range(B):
            table[r * NPAD : r * NPAD + NPC, b * D : (b + 1) * D] = h0[b, r * NPC : (r + 1) * NPC]
    return K, C, idx_all, oh_all, g0_all, table


def _build(K, C):
    nc = bacc.Bacc("TRN2", target_bir_lowering=False, debug=False, num_devices=N_CORES)
    TROWS = N_CORES * NPAD
    t_h0 = nc.dram_tensor("h0_table", [TROWS, 2 * D], F32, kind="ExternalInput")
    t_own0 = nc.dram_tensor("own0", [NPAD, 2 * D], F32, kind="ExternalInput")
    t_idx = nc.dram_tensor("idx_in", [128, C], mybir.dt.int32, kind="ExternalInput")
    t_oh = nc.dram_tensor("oh_in", [128, C * 128], BF16, kind="ExternalInput")
    t_g0 = nc.dram_tensor("g0_in", [128, C * 128], BF16, kind="ExternalInput")
    t_few = nc.dram_tensor("few", [2 * D, L * D], F32, kind="ExternalInput")
    t_feb = nc.dram_tensor("feb", [D, L], F32, kind="ExternalInput")
    t_fvw = nc.dram_tensor("fvw", [D, L * D], F32, kind="ExternalInput")
    t_fvb = nc.dram_tensor("fvb", [D, L], F32, kind="ExternalInput")
    t_out = nc.dram_tensor("hout", [NPAD, 2 * D], F32, kind="ExternalOutput")

    with tile.TileContext(nc) as tc:
        with (
            tc.tile_pool(name="per", bufs=1) as per,
            tc.tile_pool(name="work", bufs=3) as work,
            tc.tile_pool(name="pT", bufs=4, space="PSUM") as pT,
            tc.tile_pool(name="pE", bufs=2, space="PSUM") as pE,
            tc.tile_pool(name="pA", bufs=2, space="PSUM") as pA,
            tc.tile_pool(name="dram", bufs=1, space="DRAM") as dram,
        ):
            idx_sb = per.tile([128, C], mybir.dt.int32, tag="idx")
            oh_sb = per.tile([128, C * 128], BF16, tag="oh")
            g_sb = per.tile([128, C * 128], BF16, tag="g")
            ident_f = per.tile([128, 128], F32, tag="idf")
            ident_b = per.tile([128, 128], BF16, tag="idb")
            few_sb = per.tile([2 * D, L * D], F32, tag="few")
            feb_sb = per.tile([D, L], F32, tag="feb")
            fvw_sb = per.tile([D, L * D], F32, tag="fvw")
            fvb_sb = per.tile([D, L], F32, tag="fvb")

            nc.sync.dma_start(out=idx_sb[:], in_=t_idx[:])
            nc.sync.dma_start(out=oh_sb[:], in_=t_oh[:])
            nc.sync.dma_start(out=g_sb[:], in_=t_g0[:])
            nc.sync.dma_start(out=few_sb[:], in_=t_few[:])
            nc.sync.dma_start(out=feb_sb[:], in_=t_feb[:])
            nc.sync.dma_start(out=fvw_sb[:], in_=t_fvw[:])
            nc.sync.dma_start(out=fvb_sb[:], in_=t_fvb[:])
            make_identity(nc, ident_f[:])
            make_identity(nc, ident_b[:])

            cc_in = [
                dram.tile([NPAD, 2 * D], F32, tag=f"ccin{l}", name=f"ccin{l}")
                for l in range(L - 1)
            ]
            cc_out = [
                dram.tile([TROWS, 2 * D], F32, tag=f"ccout{l}", name=f"ccout{l}")
                for l in range(L - 1)
            ]

            for l in range(L):
                table = t_h0 if l == 0 else cc_out[l - 1]
                ownsrc = t_own0 if l == 0 else cc_in[l - 1]
                target = t_out if l == L - 1 else cc_in[l]
                feW = few_sb[:, l * D : (l + 1) * D]
                feb = feb_sb[:, l : l + 1]
                fvW = fvw_sb[:, l * D : (l + 1) * D]
                fvb = fvb_sb[:, l : l + 1]
                for t in range(TILES):
                    agg = [
                        pA.tile([64, 128], F32, tag="agg", name=f"agg{l}_{t}_{bb}")
                        for bb in range(B)
                    ]
                    for c in range(K):
                        gidx = t * K + c
                        sl = slice(gidx * 128, (gidx + 1) * 128)
                        gath = work.tile([128, 2 * D], F32, tag="gath")
                        nc.gpsimd.indirect_dma_start(
                            out=gath[:],
                            out_offset=None,
                            in_=table[:],
                            in_offset=IndirectOffsetOnAxis(ap=idx_sb[:, gidx : gidx + 1], axis=0),
                        )
                        for b in range(B):
                            psT = pT.tile([64, 128], F32, tag="pT")
                            nc.tensor.transpose(psT[:], gath[:, b * D : (b + 1) * D], ident_f[:])
                            cat = work.tile([128, 128], F32, tag="cat")
                            nc.vector.tensor_copy(cat[64:128, :], psT[:])
                            nc.vector.tensor_copy(cat[0:64, :], g_sb[b * 64 : b * 64 + 64, sl])
                            psE = pE.tile([64, 128], F32, tag="pE")
                            nc.tensor.matmul(psE[:], lhsT=feW, rhs=cat[:], start=True, stop=True)
                            nc.scalar.activation(
                                g_sb[b * 64 : b * 64 + 64, sl], psE[:],
                                mybir.ActivationFunctionType.Relu, bias=feb,
                            )
                            psT2 = pT.tile([128, 64], BF16, tag="pT")
                            nc.tensor.transpose(
                                psT2[:],
                                g_sb[b * 64 : b * 64 + 64, sl],
                                ident_b[b * 64 : b * 64 + 64, b * 64 : b * 64 + 64],
                            )
                            gem = work.tile([128, 64], BF16, tag="gem")
                            nc.vector.tensor_copy(gem[:], psT2[:])
                            nc.tensor.matmul(
                                agg[b][:], lhsT=gem[:], rhs=oh_sb[:, sl],
                                start=(c == 0), stop=(c == K - 1),
                            )
                    rows = slice(t * 128, (t + 1) * 128)
                    for b in range(B):
                        hv = work.tile([128, 64], F32, tag="hv")
                        nc.sync.dma_start(out=hv[:], in_=ownsrc[rows, b * D : (b + 1) * D])
                        psT = pT.tile([64, 128], F32, tag="pT")
                        nc.tensor.transpose(psT[:], hv[:], ident_f[:])
                        hvT = work.tile([64, 128], F32, tag="hvT")
                        nc.vector.tensor_copy(hvT[:], psT[:])
                        psH = pE.tile([64, 128], F32, tag="pE")
                        nc.tensor.matmul(psH[:], lhsT=fvW, rhs=hvT[:], start=True, stop=True)
                        s1 = work.tile([64, 128], F32, tag="s1")
                        nc.scalar.activation(s1[:], psH[:], mybir.ActivationFunctionType.Relu, bias=fvb)
                        s2 = work.tile([64, 128], F32, tag="s2")
                        nc.vector.tensor_tensor(s2[:], s1[:], agg[b][:], op=mybir.AluOpType.add)
                        hnT = work.tile([64, 128], F32, tag="hnT")
                        nc.scalar.activation(hnT[:], s2[:], mybir.ActivationFunctionType.Relu)
                        psT3 = pT.tile([128, 64], F32, tag="pT")
                        nc.tensor.transpose(psT3[:], hnT[:], ident_f[0:64, 0:64])
                        hn = work.tile([128, 64], F32, tag="hn")
                        nc.vector.tensor_copy(hn[:], psT3[:])
                        nc.sync.dma_start(out=target[rows, b * D : (b + 1) * D], in_=hn[:])
                if l < L - 1:
                    nc.gpsimd.collective_compute(
                        "AllGather",
                        mybir.AluOpType.bypass,
                        replica_groups=[list(range(N_CORES))],
                        ins=[cc_in[l].opt()],
                        outs=[cc_out[l].opt()],
                    )
    nc.compile()
    return nc


LAST_RESULT = None
LAST_EXEC_NS = None


def kernel(**inputs):
    x = np.asarray(inputs["x"], np.float32)
    K, C, idx_all, oh_all, g0_all, table = _host_prep(
        x, inputs["edge_index"], inputs["edge_attr"],
        inputs["pv_W"], inputs["pv_b"], inputs["pe_W"], inputs["pe_b"],
    )
    fe_W = np.asarray(inputs["fe_W"], np.float32)   # [L, 2D, D]
    fe_b = np.asarray(inputs["fe_b"], np.float32)   # [L, D]
    fv_W = np.asarray(inputs["fv_W"], np.float32)   # [L, D, D]
    fv_b = np.asarray(inputs["fv_b"], np.float32)   # [L, D]

    nc = _build(K, C)

    few = fe_W.transpose(1, 0, 2).reshape(2 * D, L * D).astype(np.float32)
    fvw = fv_W.transpose(1, 0, 2).reshape(D, L * D).astype(np.float32)
    feb = fe_b.T.copy().astype(np.float32)  # [D, L]
    fvb = fv_b.T.copy().astype(np.float32)  # [D, L]
    in_maps = []
    for r in range(N_CORES):
        in_maps.append({
            "h0_table": table,
            "own0": table[r * NPAD : (r + 1) * NPAD].copy(),
            "idx_in": idx_all[r],
            "oh_in": oh_all[r],
            "g0_in": g0_all[r],
            "few": few,
            "feb": feb,
            "fvw": fvw,
            "fvb": fvb,
        })
    res = run_bass_kernel_spmd(nc, in_maps, core_ids=list(range(N_CORES)))
    global LAST_RESULT, LAST_EXEC_NS
    LAST_RESULT = res
    import os, time as _time
    if os.environ.get("KERNEL_TIME_2ND"):
        t0 = _time.time()
        run_bass_kernel_spmd(nc, in_maps, core_ids=list(range(N_CORES)))
        LAST_EXEC_NS = int((_time.time() - t0) * 1e9)

    h_fin = np.zeros((B, N, D), np.float32)
    for r in range(N_CORES):
        o = np.asarray(res.results[r]["hout"], np.float32)
        for b in range(B):
            h_fin[b, r * NPC : (r + 1) * NPC] = o[:NPC, b * D : (b + 1) * D]

    final_W = np.asarray(inputs["final_W"], np.float32)
    final_b = np.asarray(inputs["final_b"], np.float32)
    scores = (h_fin @ final_W + final_b)[..., 0]
    susceptible = x[..., 0] > 0.5
    scores = np.where(susceptible, -np.inf, scores).astype(np.float32)
    mx = np.max(scores, axis=-1, keepdims=True)
    z = scores - mx
    lse = np.log(np.sum(np.exp(z), axis=-1, keepdims=True))
    return (z - lse).astype(np.float32)
